# revision 1
# baseline (speedup 1.0000x reference)
"""MoE layer v2: token-data-parallel + routed-expert sparsity (capacity 384).

Per core (1024 tokens): router computes top-2 combine weights and per-expert
ranks (exclusive prefix counts via triangular matmuls). For each routed
expert: a 0/1 selection matrix (built with DVE compares) matmuls out the
slot->token map and slot gatings; dma_gather pulls the ~292 selected token
rows from HBM; PE transposes give X_e^T; SwiGLU runs on 384 slots instead of
1024 tokens; results are scaled by slot gatings and dma_scatter_add'ed into
the output. The shared expert runs dense and writes output rows directly.
"""

import numpy as np
from contextlib import ExitStack

import concourse.bass as bass
import concourse.mybir as mybir
import concourse.tile as tile
from concourse import bacc
from concourse.bass_utils import run_bass_kernel_spmd

B, S, D = 4, 2048, 1024
E = 8
I = 938
IP = 1024
GU = 2 * IP
NE = E + 1
N_CORES = 8
T = (B * S) // N_CORES   # 1024 tokens/core
C = 384                  # expert capacity (max observed load 292)
CB = C // 128            # capacity chunks

P = 128
KD = D // P
KI = IP // P
MT = T // P
ND2 = D // 512

F32 = mybir.dt.float32
F32R = mybir.dt.float32r
I16 = mybir.dt.int16
AF = mybir.ActivationFunctionType
OP = mybir.AluOpType
AX = mybir.AxisListType

# shared expert token chunking (reuses the 384-wide pools)
SH_CHUNKS = [(0, 384), (384, 384), (768, 256)]


def build_moe():
    nc = bacc.Bacc("TRN2", target_bir_lowering=False, debug=False,
                   enable_asserts=True, num_devices=N_CORES)
    xT = nc.dram_tensor("xT", [D, T], F32R, kind="ExternalInput")
    xTok = nc.dram_tensor("xTok", [T, D], F32R, kind="ExternalInput")
    gwT = nc.dram_tensor("gwT", [D, E], F32, kind="ExternalInput")
    wgu = nc.dram_tensor("wgu", [NE, D, GU], F32R, kind="ExternalInput")
    wdn = nc.dram_tensor("wdn", [NE, IP, D], F32R, kind="ExternalInput")
    ident = nc.dram_tensor("ident", [P, P], F32R, kind="ExternalInput")
    triu = nc.dram_tensor("triu", [P, P], F32R, kind="ExternalInput")
    ones = nc.dram_tensor("ones", [P, P], F32R, kind="ExternalInput")
    iotaC = nc.dram_tensor("iotaC", [P, C], F32, kind="ExternalInput")
    iotaT = nc.dram_tensor("iotaT", [T], F32R, kind="ExternalInput")
    out = nc.dram_tensor("out", [T, D], F32, kind="ExternalOutput")

    with tile.TileContext(nc) as tc, ExitStack() as ctx:
        xt_pool = ctx.enter_context(tc.tile_pool(name="xt", bufs=KD))
        wgu_pool = ctx.enter_context(tc.tile_pool(name="wgu", bufs=18))
        wdn_pool = ctx.enter_context(tc.tile_pool(name="wdn", bufs=8))
        a_pool = ctx.enter_context(tc.tile_pool(name="a", bufs=8))
        xe_pool = ctx.enter_context(tc.tile_pool(name="xe", bufs=1))
        xet_pool = ctx.enter_context(tc.tile_pool(name="xet", bufs=8))
        y_pool = ctx.enter_context(tc.tile_pool(name="y", bufs=1))
        sel_pool = ctx.enter_context(tc.tile_pool(name="sel", bufs=8))
        tmp_pool = ctx.enter_context(tc.tile_pool(name="tmp", bufs=2))
        rt_pool = ctx.enter_context(tc.tile_pool(name="rt", bufs=4))
        cst_pool = ctx.enter_context(tc.tile_pool(name="cst", bufs=1))
        idx_pool = ctx.enter_context(tc.tile_pool(name="idx", bufs=2))
        dram_pool = ctx.enter_context(tc.tile_pool(name="dram", bufs=2, space="DRAM"))
        ps_g = ctx.enter_context(tc.tile_pool(name="psg", bufs=2, space="PSUM"))
        ps_u = ctx.enter_context(tc.tile_pool(name="psu", bufs=2, space="PSUM"))
        ps_o = ctx.enter_context(tc.tile_pool(name="pso", bufs=2, space="PSUM"))
        ps_t = ctx.enter_context(tc.tile_pool(name="pst", bufs=1, space="PSUM"))
        ps_s = ctx.enter_context(tc.tile_pool(name="pss", bufs=1, space="PSUM"))

        # ---- constants / X ----
        xts = []
        for k in range(KD):
            t = xt_pool.tile([P, T], F32R, tag="xt", name=f"xt{k}")
            nc.sync.dma_start(t[:], xT[k * P:(k + 1) * P, :])
            xts.append(t)
        idn = cst_pool.tile([P, P], F32R, tag="idn")
        nc.sync.dma_start(idn[:], ident[:])
        tri = cst_pool.tile([P, P], F32R, tag="tri")
        nc.sync.dma_start(tri[:], triu[:])
        one = cst_pool.tile([P, P], F32R, tag="one")
        nc.sync.dma_start(one[:], ones[:])
        ioc = cst_pool.tile([P, C], F32, tag="ioc")
        nc.sync.dma_start(ioc[:], iotaC[:])
        iot = cst_pool.tile([P, MT], F32R, tag="iot")
        nc.sync.dma_start(iot[:], bass.AP(tensor=iotaT, offset=0,
                                          ap=[[1, P], [P, MT]]))
        gwts = []
        for k in range(KD):
            g = rt_pool.tile([P, E], F32, tag="gw", bufs=KD, name=f"gw{k}")
            nc.sync.dma_start(g[:], gwT[k * P:(k + 1) * P, :])
            gwts.append(g)

        # ---- router: cw (f32r) + top2 mask (f32r) per token-chunk ----
        cw_tiles, mask_tiles = [], []
        for mt in range(MT):
            pl = ps_s.tile([P, E], F32, tag="pss", name=f"pl{mt}")
            for k in range(KD):
                nc.tensor.matmul(pl[:], xts[k][:, mt * P:(mt + 1) * P].bitcast(F32),
                                 gwts[k][:], start=(k == 0), stop=(k == KD - 1))
            m1 = rt_pool.tile([P, 1], F32, tag="m1")
            nc.vector.reduce_max(m1[:], pl[:], axis=AX.X)
            nm1 = rt_pool.tile([P, 1], F32, tag="nm1")
            nc.vector.tensor_scalar(nm1[:], m1[:], -1.0, None, op0=OP.mult)
            t1 = rt_pool.tile([P, E], F32, tag="t1")
            nc.vector.tensor_scalar(t1[:], pl[:], m1[:], None, op0=OP.is_ge)
            lm = rt_pool.tile([P, E], F32, tag="lm")
            nc.vector.scalar_tensor_tensor(lm[:], t1[:], -1e30, pl[:],
                                           op0=OP.mult, op1=OP.add)
            m2 = rt_pool.tile([P, 1], F32, tag="m2")
            nc.vector.reduce_max(m2[:], lm[:], axis=AX.X)
            el = rt_pool.tile([P, E], F32, tag="el")
            nc.scalar.activation(el[:], pl[:], AF.Exp, bias=nm1[:])
            ssum = rt_pool.tile([P, 1], F32, tag="ssum")
            nc.vector.reduce_sum(ssum[:], el[:], axis=AX.X)
            el1 = rt_pool.tile([P, 1], F32, tag="el1")
            nc.vector.reduce_max(el1[:], el[:], axis=AX.X)
            el2 = rt_pool.tile([P, 1], F32, tag="el2")
            nc.scalar.activation(el2[:], m2[:], AF.Exp, bias=nm1[:])
            den = rt_pool.tile([P, 1], F32, tag="den")
            nc.vector.tensor_tensor(den[:], el1[:], el2[:], op=OP.add)
            nc.vector.scalar_tensor_tensor(den[:], ssum[:], 1e-8, den[:],
                                           op0=OP.mult, op1=OP.add)
            rec = rt_pool.tile([P, 1], F32, tag="rec")
            nc.vector.reciprocal(rec[:], den[:])
            msk = rt_pool.tile([P, E], F32R, tag="msk", bufs=MT, name=f"msk{mt}")
            nc.vector.tensor_scalar(msk[:], pl[:], m2[:], None, op0=OP.is_ge)
            cwu = rt_pool.tile([P, E], F32, tag="cwu")
            nc.vector.tensor_tensor(cwu[:], msk[:], el[:], op=OP.mult)
            cw = rt_pool.tile([P, E], F32R, tag="cw", bufs=MT, name=f"cw{mt}")
            nc.vector.tensor_scalar(cw[:], cwu[:], rec[:], None, op0=OP.mult)
            cw_tiles.append(cw)
            mask_tiles.append(msk)

        # ---- exclusive prefix counts R[mt] [P, E] over token order ----
        r_tiles = []
        for mt in range(MT):
            pr = ps_s.tile([P, E], F32, tag="pss", name=f"pr{mt}")
            for mp in range(mt + 1):
                lhs = tri if mp == mt else one
                nc.tensor.matmul(pr[:], lhs[:], mask_tiles[mp][:],
                                 start=(mp == 0), stop=(mp == mt))
            rsb = rt_pool.tile([P, E], F32, tag="rsb", bufs=MT, name=f"rsb{mt}")
            nc.vector.tensor_copy(rsb[:], pr[:])
            r_tiles.append(rsb)

        # weights for shared expert
        def load_w(j):
            wg, wd = [], []
            for k in range(KD):
                wga = wgu_pool.tile([P, IP], F32R, tag="wgu", name=f"wga{j}_{k}")
                nc.sync.dma_start(wga[:], wgu[j, k * P:(k + 1) * P, 0:IP])
                wgb = wgu_pool.tile([P, IP], F32R, tag="wgu", name=f"wgb{j}_{k}")
                nc.sync.dma_start(wgb[:], wgu[j, k * P:(k + 1) * P, IP:GU])
                wg.append((wga, wgb))
            for k in range(KI):
                w = wdn_pool.tile([P, D], F32R, tag="wdn", name=f"wd{j}_{k}")
                nc.sync.dma_start(w[:], wdn[j, k * P:(k + 1) * P, :])
                wd.append(w)
            return wg, wd

        # prep(e): selection matrices -> slot token ids + gatings -> idxs + gather
        def prep(e):
            ex = e - 1  # routed expert index
            sels = []
            for mt in range(MT):
                rk = rt_pool.tile([P, 1], F32, tag="rk")
                nc.vector.tensor_tensor(rk[:], r_tiles[mt][:, ex:ex + 1],
                                        mask_tiles[mt][:, ex:ex + 1], op=OP.mult)
                rks = rt_pool.tile([P, 1], F32, tag="rks")
                nc.vector.scalar_tensor_tensor(rks[:], mask_tiles[mt][:, ex:ex + 1],
                                               -1.0, rk[:], op0=OP.add, op1=OP.add)
                sl = sel_pool.tile([P, C], F32R, tag="sel", name=f"sel{e}_{mt}")
                nc.vector.tensor_scalar(sl[:], ioc[:], rks[:], None, op0=OP.is_equal)
                sels.append(sl)
            tok16 = idx_pool.tile([P, CB], I16, tag="tok16", name=f"tok16_{e}")
            cws = idx_pool.tile([P, CB], F32, tag="cws", name=f"cws{e}")
            rhs2s = []
            for mt in range(MT):
                r2 = idx_pool.tile([P, 2], F32R, tag="rhs2", bufs=MT,
                                   name=f"r2_{e}_{mt}")
                nc.vector.tensor_copy(r2[:, 0:1], iot[:, mt:mt + 1])
                nc.vector.tensor_copy(r2[:, 1:2], cw_tiles[mt][:, ex:ex + 1])
                rhs2s.append(r2)
            for cb in range(CB):
                ptc = ps_s.tile([P, 2], F32, tag="pss", name=f"ptc{e}_{cb}")
                for mt in range(MT):
                    nc.tensor.matmul(ptc[:], sels[mt][:, cb * P:(cb + 1) * P],
                                     rhs2s[mt][:],
                                     start=(mt == 0), stop=(mt == MT - 1))
                nc.vector.tensor_copy(tok16[:, cb:cb + 1], ptc[:, 0:1])
                nc.vector.tensor_copy(cws[:, cb:cb + 1], ptc[:, 1:2])
            stage = dram_pool.tile([C], I16, tag="idxstage", name=f"ist{e}")
            nc.sync.dma_start(
                bass.AP(tensor=stage.tensor, offset=stage.offset,
                        ap=[[1, P], [P, CB]]), tok16[:])
            idxw = idx_pool.tile([P, C // 16], I16, tag="idxw", name=f"idxw{e}")
            for g in range(8):
                nc.sync.dma_start(
                    idxw[16 * g:16 * (g + 1), :],
                    bass.AP(tensor=stage.tensor, offset=stage.offset,
                            ap=[[1, 16], [16, C // 16]]))
            xe = xe_pool.tile([P, CB, D], F32R, tag="xe", name=f"xe{e}")
            nc.gpsimd.dma_gather(xe[:], xTok[:], idxw[:], num_idxs=C,
                                 num_idxs_reg=C, elem_size=D)
            return xe, idxw, cws

        # stage1+stage2 over a token block of width w
        def swiglu_block(wg, wd, rhs_tiles, w, emit_mid=None):
            """rhs_tiles: 8 k-tiles, use [:, 0:w]. Returns list of A tiles."""
            ats = []
            for m in range(KI):
                pg = ps_g.tile([P, 384], F32, tag="psg", name=f"pg{m}")
                for k in range(KD):
                    nc.tensor.matmul(pg[:, 0:w], wg[k][0][:, m * P:(m + 1) * P],
                                     rhs_tiles[k][:, 0:w],
                                     start=(k == 0), stop=(k == KD - 1))
                pu = ps_u.tile([P, 384], F32, tag="psu", name=f"pu{m}")
                for k in range(KD):
                    nc.tensor.matmul(pu[:, 0:w], wg[k][1][:, m * P:(m + 1) * P],
                                     rhs_tiles[k][:, 0:w],
                                     start=(k == 0), stop=(k == KD - 1))
                st = tmp_pool.tile([P, 512], F32, tag="tmp", name=f"st{m}")
                nc.scalar.activation(st[:, 0:w], pg[:, 0:w], AF.Silu)
                at = a_pool.tile([P, C], F32R, tag="a", name=f"a{m}")
                nc.vector.tensor_tensor(at[:, 0:w], st[:, 0:w], pu[:, 0:w], op=OP.mult)
                ats.append(at)
                if emit_mid is not None and m == 3:
                    emit_mid()
            return ats

        # ---- shared expert (dense over all tokens, direct output writes) ----
        wg0, wd0 = load_w(0)
        pending = {}
        pending[1] = prep(1)

        for tc_i, (off, w) in enumerate(SH_CHUNKS):
            def rhs_slices():
                return [xts[k][:, off:off + w] for k in range(KD)]
            # build temp AP views with token offset
            rhs = [xts[k][:, off:off + w] for k in range(KD)]
            # wrap as objects exposing [:, 0:w]
            class _V:
                def __init__(self, ap):
                    self.ap_ = ap
                def __getitem__(self, sl):
                    return self.ap_
            ats = swiglu_block(wg0, wd0, [_V(r) for r in rhs], w)
            for sub in range(w // P):
                for nd in range(ND2):
                    po = ps_o.tile([P, 512], F32, tag="pso", name=f"spo{tc_i}_{sub}_{nd}")
                    for k in range(KI):
                        nc.tensor.matmul(po[:], ats[k][:, sub * P:(sub + 1) * P],
                                         wd0[k][:, nd * 512:(nd + 1) * 512],
                                         start=(k == 0), stop=(k == KI - 1))
                    ot = tmp_pool.tile([P, 512], F32, tag="tmp", name=f"so{tc_i}_{sub}_{nd}")
                    nc.vector.tensor_copy(ot[:], po[:])
                    rows = off + sub * P
                    nc.sync.dma_start(out[rows:rows + P, nd * 512:(nd + 1) * 512], ot[:])

        # ---- routed experts ----
        for e in range(1, NE):
            xe, idxw, cws = pending.pop(e)
            wg, wd = load_w(e)
            # transpose gathered tokens -> X_e^T
            xet = []
            for db in range(KD):
                xt_t = xet_pool.tile([P, C], F32R, tag="xet", name=f"xet{e}_{db}")
                xet.append(xt_t)
            for cb in range(CB):
                for db in range(KD):
                    pt = ps_t.tile([P, P], F32R, tag="pst", name=f"tp{e}_{cb}_{db}")
                    nc.tensor.transpose(pt[:], xe[:, cb, db * P:(db + 1) * P], idn[:])
                    nc.vector.tensor_copy(xet[db][:, cb * P:(cb + 1) * P], pt[:])

            def emit_next_prep():
                if e + 1 < NE and (e + 1) not in pending:
                    pending[e + 1] = prep(e + 1)

            ats = swiglu_block(wg, wd, xet, C, emit_mid=emit_next_prep)

            ysb = y_pool.tile([P, CB, D], F32, tag="y", name=f"y{e}")
            for cb in range(CB):
                for nd in range(ND2):
                    po = ps_o.tile([P, 512], F32, tag="pso", name=f"po{e}_{cb}_{nd}")
                    for k in range(KI):
                        nc.tensor.matmul(po[:], ats[k][:, cb * P:(cb + 1) * P],
                                         wd[k][:, nd * 512:(nd + 1) * 512],
                                         start=(k == 0), stop=(k == KI - 1))
                    nc.vector.tensor_scalar(ysb[:, cb, nd * 512:(nd + 1) * 512],
                                            po[:], cws[:, cb:cb + 1], None,
                                            op0=OP.mult)
            nc.gpsimd.dma_scatter_add(out[:], ysb[:], idxw[:], num_idxs=C,
                                      num_idxs_reg=C, elem_size=D)

    nc.compile()
    return nc


_NC_CACHE = None


def _get_nc():
    global _NC_CACHE
    if _NC_CACHE is None:
        _NC_CACHE = build_moe()
    return _NC_CACHE


def _prep_weights(gate_weight, shared_gate_up, shared_down,
                  experts_gate_up, experts_down):
    wgu = np.zeros((NE, D, GU), np.float32)
    wgu[0, :, 0:I] = shared_gate_up[0:I].T
    wgu[0, :, IP:IP + I] = shared_gate_up[I:2 * I].T
    for e in range(E):
        wgu[e + 1, :, 0:I] = experts_gate_up[e, 0:I].T
        wgu[e + 1, :, IP:IP + I] = experts_gate_up[e, I:2 * I].T
    wdn = np.zeros((NE, IP, D), np.float32)
    wdn[0, 0:I, :] = shared_down.T
    for e in range(E):
        wdn[e + 1, 0:I, :] = experts_down[e].T
    gwT = np.ascontiguousarray(gate_weight.T.astype(np.float32))
    return gwT, np.ascontiguousarray(wgu), np.ascontiguousarray(wdn)


def _consts():
    return {
        "ident": np.eye(P, dtype=np.float32),
        "triu": np.triu(np.ones((P, P), np.float32), 1),
        "ones": np.ones((P, P), np.float32),
        "iotaC": np.broadcast_to(np.arange(C, dtype=np.float32), (P, C)).copy(),
        "iotaT": np.arange(T, dtype=np.float32),
    }


def make_in_maps(hidden_states, gate_weight, shared_gate_up, shared_down,
                 experts_gate_up, experts_down):
    hidden_states = np.asarray(hidden_states, dtype=np.float32)
    x = hidden_states.reshape(B * S, D)
    gwT, wgu, wdn = _prep_weights(
        np.asarray(gate_weight, np.float32),
        np.asarray(shared_gate_up, np.float32),
        np.asarray(shared_down, np.float32),
        np.asarray(experts_gate_up, np.float32),
        np.asarray(experts_down, np.float32))
    consts = _consts()
    in_maps = []
    for c in range(N_CORES):
        xs = np.ascontiguousarray(x[c * T:(c + 1) * T])
        in_maps.append({
            "xT": np.ascontiguousarray(xs.T), "xTok": xs,
            "gwT": gwT, "wgu": wgu, "wdn": wdn, **consts,
        })
    return in_maps


def kernel(hidden_states, gate_weight, shared_gate_up, shared_down,
           experts_gate_up, experts_down):
    in_maps = make_in_maps(hidden_states, gate_weight, shared_gate_up,
                           shared_down, experts_gate_up, experts_down)
    nc = _get_nc()
    res = run_bass_kernel_spmd(nc, in_maps, core_ids=list(range(N_CORES)))
    out = np.concatenate([res.results[c]["out"] for c in range(N_CORES)], axis=0)
    return out.reshape(B, S, D)



# revision 3
# speedup vs baseline: 1.7846x; 1.7846x over previous
"""MoE layer v3: token-data-parallel, capacity-384 routed experts, bf16 compute.

Per core (1024 tokens): router computes top-2 combine weights in fp32
(flipped matmul: gate weights stationary -> logits^T, then 8 small PE
transposes back to token-major). Per-expert slot ranks come from triangular
prefix matmuls (f32r); the slot->token map and slot gatings come from one
accumulated [2,C] matmul (rhs2 stationary). dma_gather(transpose=True)
pulls the selected token rows from HBM directly in X^T layout (bf16), so
SwiGLU needs no PE transposes. All expert/shared matmuls run in bf16
(fp32 PSUM accumulate); weights are double-buffered so the next expert's
HBM loads hide under the current expert's compute. Results are scaled by
slot gatings and dma_scatter_add'ed into the fp32 output; the shared
expert runs dense and writes output rows directly.
"""

import numpy as np
from contextlib import ExitStack

import ml_dtypes

import concourse.bass as bass
import concourse.mybir as mybir
import concourse.tile as tile
from concourse import bacc
from concourse.bass_utils import run_bass_kernel_spmd

B, S, D = 4, 2048, 1024
E = 8
I = 938
IP = 1024
GU = 2 * IP
NE = E + 1
N_CORES = 8
T = (B * S) // N_CORES   # 1024 tokens/core
C = 384                  # expert capacity (max observed load 292)
CB = C // 128            # capacity chunks

P = 128
KD = D // P
KI = IP // P
MT = T // P
ND2 = D // 512

F32 = mybir.dt.float32
F32R = mybir.dt.float32r
BF = mybir.dt.bfloat16
I16 = mybir.dt.int16
AF = mybir.ActivationFunctionType
OP = mybir.AluOpType
AX = mybir.AxisListType

# shared expert token chunking (bf16 moving operand, 512-wide PSUM)
SH_CHUNKS = [(0, 512), (512, 512)]


def build_moe():
    nc = bacc.Bacc("TRN2", target_bir_lowering=False, debug=False,
                   enable_asserts=True, num_devices=N_CORES)
    xTf = nc.dram_tensor("xTf", [D, T], F32, kind="ExternalInput")
    xTb = nc.dram_tensor("xTb", [D, T], BF, kind="ExternalInput")
    xTok = nc.dram_tensor("xTok", [T, D], BF, kind="ExternalInput")
    gwT = nc.dram_tensor("gwT", [D, E], F32, kind="ExternalInput")
    wgu = nc.dram_tensor("wgu", [NE, D, GU], BF, kind="ExternalInput")
    wdn = nc.dram_tensor("wdn", [NE, IP, D], BF, kind="ExternalInput")
    ident = nc.dram_tensor("ident", [P, P], F32, kind="ExternalInput")
    triu = nc.dram_tensor("triu", [P, P], F32R, kind="ExternalInput")
    ones = nc.dram_tensor("ones", [P, P], F32R, kind="ExternalInput")
    iotaC = nc.dram_tensor("iotaC", [P, C], F32, kind="ExternalInput")
    iotaT = nc.dram_tensor("iotaT", [T], F32R, kind="ExternalInput")
    out = nc.dram_tensor("out", [T, D], F32, kind="ExternalOutput")

    with tile.TileContext(nc) as tc, ExitStack() as ctx:
        xf_pool = ctx.enter_context(tc.tile_pool(name="xf", bufs=KD))
        xb_pool = ctx.enter_context(tc.tile_pool(name="xb", bufs=KD))
        wgu_pool = ctx.enter_context(tc.tile_pool(name="wgu", bufs=32))
        wdn_pool = ctx.enter_context(tc.tile_pool(name="wdn", bufs=16))
        a_pool = ctx.enter_context(tc.tile_pool(name="a", bufs=8))
        xet_pool = ctx.enter_context(tc.tile_pool(name="xet", bufs=2))
        y_pool = ctx.enter_context(tc.tile_pool(name="y", bufs=1))
        sel_pool = ctx.enter_context(tc.tile_pool(name="sel", bufs=8))
        tmp_pool = ctx.enter_context(tc.tile_pool(name="tmp", bufs=2))
        rt_pool = ctx.enter_context(tc.tile_pool(name="rt", bufs=4))
        cst_pool = ctx.enter_context(tc.tile_pool(name="cst", bufs=1))
        idx_pool = ctx.enter_context(tc.tile_pool(name="idx", bufs=2))
        dram_pool = ctx.enter_context(tc.tile_pool(name="dram", bufs=4, space="DRAM"))
        ps_g = ctx.enter_context(tc.tile_pool(name="psg", bufs=2, space="PSUM"))
        ps_u = ctx.enter_context(tc.tile_pool(name="psu", bufs=2, space="PSUM"))
        ps_o = ctx.enter_context(tc.tile_pool(name="pso", bufs=2, space="PSUM"))
        ps_s = ctx.enter_context(tc.tile_pool(name="pss", bufs=2, space="PSUM"))

        # ---- X (fp32 for router, bf16 for shared expert) + constants ----
        xfs = []
        for k in range(KD):
            t = xf_pool.tile([P, T], F32, tag="xf", name=f"xf{k}")
            nc.sync.dma_start(t[:], xTf[k * P:(k + 1) * P, :])
            xfs.append(t)
        gwts = []
        for k in range(KD):
            g = rt_pool.tile([P, E], F32, tag="gw", bufs=KD, name=f"gw{k}")
            nc.sync.dma_start(g[:], gwT[k * P:(k + 1) * P, :])
            gwts.append(g)
        xbs = []
        for k in range(KD):
            t = xb_pool.tile([P, T], BF, tag="xb", name=f"xb{k}")
            nc.sync.dma_start(t[:], xTb[k * P:(k + 1) * P, :])
            xbs.append(t)
        idn = cst_pool.tile([P, P], F32, tag="idn")
        nc.sync.dma_start(idn[:], ident[:])
        tri = cst_pool.tile([P, P], F32R, tag="tri")
        nc.sync.dma_start(tri[:], triu[:])
        one = cst_pool.tile([P, P], F32R, tag="one")
        nc.sync.dma_start(one[:], ones[:])
        ioc = cst_pool.tile([P, C], F32, tag="ioc")
        nc.sync.dma_start(ioc[:], iotaC[:])
        iot = cst_pool.tile([P, MT], F32R, tag="iot")
        nc.sync.dma_start(iot[:], bass.AP(tensor=iotaT, offset=0,
                                          ap=[[1, P], [P, MT]]))

        # weights (bf16): gate/up halves + down proj for expert slot j
        def load_w(j):
            wg, wd = [], []
            for k in range(KD):
                wga = wgu_pool.tile([P, IP], BF, tag="wgu", name=f"wga{j}_{k}")
                nc.sync.dma_start(wga[:], wgu[j, k * P:(k + 1) * P, 0:IP])
                wgb = wgu_pool.tile([P, IP], BF, tag="wgu", name=f"wgb{j}_{k}")
                nc.sync.dma_start(wgb[:], wgu[j, k * P:(k + 1) * P, IP:GU])
                wg.append((wga, wgb))
            for k in range(KI):
                w = wdn_pool.tile([P, D], BF, tag="wdn", name=f"wd{j}_{k}")
                nc.sync.dma_start(w[:], wdn[j, k * P:(k + 1) * P, :])
                wd.append(w)
            return wg, wd

        wts = {0: load_w(0), 1: load_w(1)}

        # ---- router (fp32): logits^T = gwT.T @ x, transpose back per chunk ----
        lps = []
        for h in range(2):
            pl = ps_s.tile([P, 512], F32, tag="pss", name=f"plT{h}")
            for k in range(KD):
                nc.tensor.matmul(pl[0:E, :], gwts[k][:],
                                 xfs[k][:, h * 512:(h + 1) * 512],
                                 start=(k == 0), stop=(k == KD - 1))
            lps.append(pl)
        lgT = rt_pool.tile([E, T], F32, tag="lgT", bufs=1)
        for h in range(2):
            nc.vector.tensor_copy(lgT[0:E, h * 512:(h + 1) * 512], lps[h][0:E, :])

        cw_tiles, mask_tiles = [], []
        for mt in range(MT):
            pt = ps_s.tile([P, 512], F32, tag="pss", name=f"plt{mt}")
            nc.tensor.transpose(pt[0:P, 0:E], lgT[0:E, mt * P:(mt + 1) * P],
                                idn[0:E, 0:E])
            pl = rt_pool.tile([P, E], F32, tag="pl", bufs=2, name=f"pl{mt}")
            nc.vector.tensor_copy(pl[:], pt[0:P, 0:E])
            m1 = rt_pool.tile([P, 1], F32, tag="m1")
            nc.vector.reduce_max(m1[:], pl[:], axis=AX.X)
            nm1 = rt_pool.tile([P, 1], F32, tag="nm1")
            nc.vector.tensor_scalar(nm1[:], m1[:], -1.0, None, op0=OP.mult)
            t1 = rt_pool.tile([P, E], F32, tag="t1")
            nc.vector.tensor_scalar(t1[:], pl[:], m1[:], None, op0=OP.is_ge)
            lm = rt_pool.tile([P, E], F32, tag="lm")
            nc.vector.scalar_tensor_tensor(lm[:], t1[:], -1e30, pl[:],
                                           op0=OP.mult, op1=OP.add)
            m2 = rt_pool.tile([P, 1], F32, tag="m2")
            nc.vector.reduce_max(m2[:], lm[:], axis=AX.X)
            el = rt_pool.tile([P, E], F32, tag="el")
            nc.scalar.activation(el[:], pl[:], AF.Exp, bias=nm1[:])
            ssum = rt_pool.tile([P, 1], F32, tag="ssum")
            nc.vector.reduce_sum(ssum[:], el[:], axis=AX.X)
            el1 = rt_pool.tile([P, 1], F32, tag="el1")
            nc.vector.reduce_max(el1[:], el[:], axis=AX.X)
            el2 = rt_pool.tile([P, 1], F32, tag="el2")
            nc.scalar.activation(el2[:], m2[:], AF.Exp, bias=nm1[:])
            den = rt_pool.tile([P, 1], F32, tag="den")
            nc.vector.tensor_tensor(den[:], el1[:], el2[:], op=OP.add)
            nc.vector.scalar_tensor_tensor(den[:], ssum[:], 1e-8, den[:],
                                           op0=OP.mult, op1=OP.add)
            rec = rt_pool.tile([P, 1], F32, tag="rec")
            nc.vector.reciprocal(rec[:], den[:])
            msk = rt_pool.tile([P, E], F32R, tag="msk", bufs=MT, name=f"msk{mt}")
            nc.vector.tensor_scalar(msk[:], pl[:], m2[:], None, op0=OP.is_ge)
            cwu = rt_pool.tile([P, E], F32, tag="cwu")
            nc.vector.tensor_tensor(cwu[:], msk[:], el[:], op=OP.mult)
            cw = rt_pool.tile([P, E], F32R, tag="cw", bufs=MT, name=f"cw{mt}")
            nc.vector.tensor_scalar(cw[:], cwu[:], rec[:], None, op0=OP.mult)
            cw_tiles.append(cw)
            mask_tiles.append(msk)

        # ---- exclusive prefix counts R[mt] [P, E] over token order ----
        r_tiles = []
        for mt in range(MT):
            pr = ps_s.tile([P, 512], F32, tag="pss", name=f"pr{mt}")
            for mp in range(mt + 1):
                lhs = tri if mp == mt else one
                nc.tensor.matmul(pr[0:P, 0:E], lhs[:], mask_tiles[mp][:],
                                 start=(mp == 0), stop=(mp == mt))
            rsb = rt_pool.tile([P, E], F32, tag="rsb", bufs=MT, name=f"rsb{mt}")
            nc.vector.tensor_copy(rsb[:], pr[0:P, 0:E])
            r_tiles.append(rsb)

        # prep(e): slot->token map + slot gatings (one [2,C] matmul), then
        # transposed dma_gather pulls X_e^T (bf16) directly.
        def prep(e):
            ex = e - 1
            sels = []
            for mt in range(MT):
                rk = rt_pool.tile([P, 1], F32, tag="rk")
                nc.vector.tensor_tensor(rk[:], r_tiles[mt][:, ex:ex + 1],
                                        mask_tiles[mt][:, ex:ex + 1], op=OP.mult)
                rks = rt_pool.tile([P, 1], F32, tag="rks")
                nc.vector.scalar_tensor_tensor(rks[:], mask_tiles[mt][:, ex:ex + 1],
                                               -1.0, rk[:], op0=OP.add, op1=OP.add)
                sl = sel_pool.tile([P, C], F32R, tag="sel", name=f"sel{e}_{mt}")
                nc.vector.tensor_scalar(sl[:], ioc[:], rks[:], None, op0=OP.is_equal)
                sels.append(sl)
            rhs2s = []
            for mt in range(MT):
                r2 = idx_pool.tile([P, 2], F32R, tag="rhs2", bufs=MT,
                                   name=f"r2_{e}_{mt}")
                nc.vector.tensor_copy(r2[:, 0:1], iot[:, mt:mt + 1])
                nc.vector.tensor_copy(r2[:, 1:2], cw_tiles[mt][:, ex:ex + 1])
                rhs2s.append(r2)
            ptc = ps_s.tile([P, 512], F32, tag="pss", name=f"ptc{e}")
            for mt in range(MT):
                nc.tensor.matmul(ptc[0:2, 0:C], rhs2s[mt][:], sels[mt][:],
                                 start=(mt == 0), stop=(mt == MT - 1))
            tokrow = idx_pool.tile([2, C], I16, tag="tokrow", name=f"tok{e}")
            nc.vector.tensor_copy(tokrow[0:2, :], ptc[0:2, 0:C])
            cwrow = idx_pool.tile([2, C], F32, tag="cwrow", name=f"cwr{e}")
            nc.vector.tensor_copy(cwrow[0:2, :], ptc[0:2, 0:C])
            stage = dram_pool.tile([C], I16, tag="idxstage", name=f"ist{e}")
            nc.sync.dma_start(
                bass.AP(tensor=stage.tensor, offset=stage.offset,
                        ap=[[C, 1], [1, C]]), tokrow[0:1, :])
            cstage = dram_pool.tile([C], F32, tag="cwstage", name=f"cst{e}")
            nc.sync.dma_start(
                bass.AP(tensor=cstage.tensor, offset=cstage.offset,
                        ap=[[C, 1], [1, C]]), cwrow[1:2, :])
            idxw = idx_pool.tile([P, C // 16], I16, tag="idxw", name=f"idxw{e}")
            for g in range(8):
                nc.sync.dma_start(
                    idxw[16 * g:16 * (g + 1), :],
                    bass.AP(tensor=stage.tensor, offset=stage.offset,
                            ap=[[1, 16], [16, C // 16]]))
            cws = idx_pool.tile([P, CB], F32, tag="cws", name=f"cws{e}")
            nc.sync.dma_start(cws[:],
                              bass.AP(tensor=cstage.tensor, offset=cstage.offset,
                                      ap=[[1, P], [P, CB]]))
            xet = xet_pool.tile([P, KD, C], BF, tag="xet", name=f"xet{e}")
            nc.gpsimd.dma_gather(xet[:], xTok[:], idxw[:], num_idxs=C,
                                 num_idxs_reg=C, elem_size=D, transpose=True)
            return xet, idxw, cws

        # stage1 over a token block of width w; rhs_fn(k) -> [P, >=w] AP
        def swiglu_block(wg, rhs_fn, w, emit_mid=None):
            ats = []
            for m in range(KI):
                pg = ps_g.tile([P, 512], F32, tag="psg", name=f"pg{m}")
                for k in range(KD):
                    nc.tensor.matmul(pg[:, 0:w], wg[k][0][:, m * P:(m + 1) * P],
                                     rhs_fn(k, w),
                                     start=(k == 0), stop=(k == KD - 1))
                pu = ps_u.tile([P, 512], F32, tag="psu", name=f"pu{m}")
                for k in range(KD):
                    nc.tensor.matmul(pu[:, 0:w], wg[k][1][:, m * P:(m + 1) * P],
                                     rhs_fn(k, w),
                                     start=(k == 0), stop=(k == KD - 1))
                st = tmp_pool.tile([P, 512], F32, tag="tmp", name=f"st{m}")
                nc.scalar.activation(st[:, 0:w], pg[:, 0:w], AF.Silu)
                at = a_pool.tile([P, 512], BF, tag="a", name=f"a{m}")
                nc.vector.tensor_tensor(at[:, 0:w], st[:, 0:w], pu[:, 0:w],
                                        op=OP.mult)
                ats.append(at)
                if emit_mid is not None and m == 3:
                    emit_mid()
            return ats

        # ---- shared expert (dense over all tokens, direct output writes) ----
        wg0, wd0 = wts.pop(0)
        pending = {1: prep(1)}

        for tc_i, (off, w) in enumerate(SH_CHUNKS):
            ats = swiglu_block(wg0, lambda k, ww: xbs[k][:, off:off + ww], w)
            for sub in range(w // P):
                for nd in range(ND2):
                    po = ps_o.tile([P, 512], F32, tag="pso",
                                   name=f"spo{tc_i}_{sub}_{nd}")
                    for k in range(KI):
                        nc.tensor.matmul(po[:], ats[k][:, sub * P:(sub + 1) * P],
                                         wd0[k][:, nd * 512:(nd + 1) * 512],
                                         start=(k == 0), stop=(k == KI - 1))
                    ot = tmp_pool.tile([P, 512], F32, tag="tmp",
                                       name=f"so{tc_i}_{sub}_{nd}")
                    nc.vector.tensor_copy(ot[:], po[:])
                    rows = off + sub * P
                    nc.sync.dma_start(out[rows:rows + P, nd * 512:(nd + 1) * 512],
                                      ot[:])

        # ---- routed experts ----
        for e in range(1, NE):
            xet, idxw, cws = pending.pop(e)
            wg, wd = wts.pop(e)
            if e + 1 < NE:
                wts[e + 1] = load_w(e + 1)

            def emit_next_prep():
                if e + 1 < NE and (e + 1) not in pending:
                    pending[e + 1] = prep(e + 1)

            ats = swiglu_block(wg, lambda k, ww: xet[:, k, 0:ww], C,
                               emit_mid=emit_next_prep)

            ysb = y_pool.tile([P, CB, D], F32, tag="y", name=f"y{e}")
            for cb in range(CB):
                for nd in range(ND2):
                    po = ps_o.tile([P, 512], F32, tag="pso",
                                   name=f"po{e}_{cb}_{nd}")
                    for k in range(KI):
                        nc.tensor.matmul(po[:], ats[k][:, cb * P:(cb + 1) * P],
                                         wd[k][:, nd * 512:(nd + 1) * 512],
                                         start=(k == 0), stop=(k == KI - 1))
                    nc.vector.tensor_scalar(ysb[:, cb, nd * 512:(nd + 1) * 512],
                                            po[:], cws[:, cb:cb + 1], None,
                                            op0=OP.mult)
            nc.gpsimd.dma_scatter_add(out[:], ysb[:], idxw[:], num_idxs=C,
                                      num_idxs_reg=C, elem_size=D)

    nc.compile()
    return nc


_NC_CACHE = None


def _get_nc():
    global _NC_CACHE
    if _NC_CACHE is None:
        _NC_CACHE = build_moe()
    return _NC_CACHE


def _prep_weights(gate_weight, shared_gate_up, shared_down,
                  experts_gate_up, experts_down):
    wgu = np.zeros((NE, D, GU), np.float32)
    wgu[0, :, 0:I] = shared_gate_up[0:I].T
    wgu[0, :, IP:IP + I] = shared_gate_up[I:2 * I].T
    for e in range(E):
        wgu[e + 1, :, 0:I] = experts_gate_up[e, 0:I].T
        wgu[e + 1, :, IP:IP + I] = experts_gate_up[e, I:2 * I].T
    wdn = np.zeros((NE, IP, D), np.float32)
    wdn[0, 0:I, :] = shared_down.T
    for e in range(E):
        wdn[e + 1, 0:I, :] = experts_down[e].T
    gwT = np.ascontiguousarray(gate_weight.T.astype(np.float32))
    BF_NP = ml_dtypes.bfloat16
    return gwT, wgu.astype(BF_NP), wdn.astype(BF_NP)


def _consts():
    return {
        "ident": np.eye(P, dtype=np.float32),
        "triu": np.triu(np.ones((P, P), np.float32), 1),
        "ones": np.ones((P, P), np.float32),
        "iotaC": np.broadcast_to(np.arange(C, dtype=np.float32), (P, C)).copy(),
        "iotaT": np.arange(T, dtype=np.float32),
    }


def make_in_maps(hidden_states, gate_weight, shared_gate_up, shared_down,
                 experts_gate_up, experts_down):
    hidden_states = np.asarray(hidden_states, dtype=np.float32)
    x = hidden_states.reshape(B * S, D)
    gwT, wgu, wdn = _prep_weights(
        np.asarray(gate_weight, np.float32),
        np.asarray(shared_gate_up, np.float32),
        np.asarray(shared_down, np.float32),
        np.asarray(experts_gate_up, np.float32),
        np.asarray(experts_down, np.float32))
    consts = _consts()
    BF_NP = ml_dtypes.bfloat16
    in_maps = []
    for c in range(N_CORES):
        xs = np.ascontiguousarray(x[c * T:(c + 1) * T])
        xsT = np.ascontiguousarray(xs.T)
        in_maps.append({
            "xTf": xsT, "xTb": xsT.astype(BF_NP), "xTok": xs.astype(BF_NP),
            "gwT": gwT, "wgu": wgu, "wdn": wdn, **consts,
        })
    return in_maps


def kernel(hidden_states, gate_weight, shared_gate_up, shared_down,
           experts_gate_up, experts_down):
    in_maps = make_in_maps(hidden_states, gate_weight, shared_gate_up,
                           shared_down, experts_gate_up, experts_down)
    nc = _get_nc()
    res = run_bass_kernel_spmd(nc, in_maps, core_ids=list(range(N_CORES)))
    out = np.concatenate([res.results[c]["out"] for c in range(N_CORES)], axis=0)
    return out.reshape(B, S, D)


# revision 10
# speedup vs baseline: 1.9048x; 1.0673x over previous
"""MoE layer v3: token-data-parallel, capacity-384 routed experts, bf16 compute.

Per core (1024 tokens): router computes top-2 combine weights in fp32
(flipped matmul: gate weights stationary -> logits^T, then 8 small PE
transposes back to token-major). Per-expert slot ranks come from triangular
prefix matmuls (f32r); the slot->token map and slot gatings come from one
accumulated [2,C] matmul (rhs2 stationary). dma_gather(transpose=True)
pulls the selected token rows from HBM directly in X^T layout (bf16), so
SwiGLU needs no PE transposes. All expert/shared matmuls run in bf16
(fp32 PSUM accumulate); weights are double-buffered so the next expert's
HBM loads hide under the current expert's compute. Results are scaled by
slot gatings and dma_scatter_add'ed into the fp32 output; the shared
expert runs dense and writes output rows directly.
"""

import numpy as np
from contextlib import ExitStack

import ml_dtypes

import concourse.bass as bass
import concourse.mybir as mybir
import concourse.tile as tile
from concourse import bacc
from concourse.bass_utils import run_bass_kernel_spmd

B, S, D = 4, 2048, 1024
E = 8
I = 938
IP = 1024
GU = 2 * IP
NE = E + 1
N_CORES = 8
T = (B * S) // N_CORES   # 1024 tokens/core
C = 384                  # gather capacity (transpose dma_gather needs %128)
CC = 320                 # compute capacity (max observed load 292)
CB = C // 128            # capacity chunks
CC_CHUNKS = [128, 128, 64]  # stage2 chunk widths covering CC slots

P = 128
KD = D // P
KI = IP // P
MT = T // P
ND2 = D // 512

F32 = mybir.dt.float32
F32R = mybir.dt.float32r
BF = mybir.dt.bfloat16
I16 = mybir.dt.int16
AF = mybir.ActivationFunctionType
OP = mybir.AluOpType
AX = mybir.AxisListType

# shared expert token chunking (bf16 moving operand, 512-wide PSUM)
SH_CHUNKS = [(0, 512), (512, 512)]


def build_moe():
    nc = bacc.Bacc("TRN2", target_bir_lowering=False, debug=False,
                   enable_asserts=True, num_devices=N_CORES)
    xTf = nc.dram_tensor("xTf", [D, T], F32, kind="ExternalInput")
    xTok = nc.dram_tensor("xTok", [T, D], BF, kind="ExternalInput")
    gwT = nc.dram_tensor("gwT", [D, E], F32, kind="ExternalInput")
    wgu = nc.dram_tensor("wgu", [NE, D, GU], BF, kind="ExternalInput")
    wdn = nc.dram_tensor("wdn", [NE, IP, D], BF, kind="ExternalInput")
    ident = nc.dram_tensor("ident", [P, P], F32, kind="ExternalInput")
    triu = nc.dram_tensor("triu", [P, P], F32R, kind="ExternalInput")
    ones = nc.dram_tensor("ones", [P, P], F32R, kind="ExternalInput")
    iotaC = nc.dram_tensor("iotaC", [P, C], F32, kind="ExternalInput")
    iotaT = nc.dram_tensor("iotaT", [T], F32R, kind="ExternalInput")
    out = nc.dram_tensor("out", [T, D], F32, kind="ExternalOutput")

    with tile.TileContext(nc) as tc, ExitStack() as ctx:
        xf_pool = ctx.enter_context(tc.tile_pool(name="xf", bufs=KD))
        xb_pool = ctx.enter_context(tc.tile_pool(name="xb", bufs=KD))
        wgu_pool = ctx.enter_context(tc.tile_pool(name="wgu", bufs=32))
        wdn_pool = ctx.enter_context(tc.tile_pool(name="wdn", bufs=16))
        a_pool = ctx.enter_context(tc.tile_pool(name="a", bufs=8))
        xet_pool = ctx.enter_context(tc.tile_pool(name="xet", bufs=2))
        y_pool = ctx.enter_context(tc.tile_pool(name="y", bufs=1))
        sel_pool = ctx.enter_context(tc.tile_pool(name="sel", bufs=4))
        tmp_pool = ctx.enter_context(tc.tile_pool(name="tmp", bufs=2))
        rt_pool = ctx.enter_context(tc.tile_pool(name="rt", bufs=4))
        cst_pool = ctx.enter_context(tc.tile_pool(name="cst", bufs=1))
        idx_pool = ctx.enter_context(tc.tile_pool(name="idx", bufs=3))
        dram_pool = ctx.enter_context(tc.tile_pool(name="dram", bufs=6, space="DRAM"))
        ps_g = ctx.enter_context(tc.tile_pool(name="psg", bufs=2, space="PSUM"))
        ps_u = ctx.enter_context(tc.tile_pool(name="psu", bufs=2, space="PSUM"))
        ps_o = ctx.enter_context(tc.tile_pool(name="pso", bufs=2, space="PSUM"))
        ps_s = ctx.enter_context(tc.tile_pool(name="pss", bufs=2, space="PSUM"))

        # ---- X (fp32 for router, bf16 for shared expert) + constants ----
        xfs = []
        for k in range(KD):
            t = xf_pool.tile([P, T], F32, tag="xf", name=f"xf{k}")
            nc.sync.dma_start(t[:], xTf[k * P:(k + 1) * P, :])
            xfs.append(t)
        gwts = []
        for k in range(KD):
            g = rt_pool.tile([P, E], F32, tag="gw", bufs=KD, name=f"gw{k}")
            nc.sync.dma_start(g[:], gwT[k * P:(k + 1) * P, :])
            gwts.append(g)
        xbs = []
        for k in range(KD):
            t = xb_pool.tile([P, T], BF, tag="xb", name=f"xb{k}")
            nc.scalar.activation(t[:], xfs[k][:], AF.Copy)
            xbs.append(t)
        idn = cst_pool.tile([P, P], F32, tag="idn")
        nc.sync.dma_start(idn[:], ident[:])
        tri = cst_pool.tile([P, P], F32R, tag="tri")
        nc.sync.dma_start(tri[:], triu[:])
        one = cst_pool.tile([P, P], F32R, tag="one")
        nc.sync.dma_start(one[:], ones[:])
        ioc = cst_pool.tile([P, C], F32, tag="ioc")
        nc.sync.dma_start(ioc[:], iotaC[:])
        iot = cst_pool.tile([P, MT], F32R, tag="iot")
        nc.sync.dma_start(iot[:], bass.AP(tensor=iotaT, offset=0,
                                          ap=[[1, P], [P, MT]]))

        # weights (bf16): gate/up halves + down proj for expert slot j
        def load_w(j):
            wg, wd = [], []
            for k in range(KD):
                wga = wgu_pool.tile([P, IP], BF, tag="wgu", name=f"wga{j}_{k}")
                nc.sync.dma_start(wga[:], wgu[j, k * P:(k + 1) * P, 0:IP])
                wgb = wgu_pool.tile([P, IP], BF, tag="wgu", name=f"wgb{j}_{k}")
                nc.sync.dma_start(wgb[:], wgu[j, k * P:(k + 1) * P, IP:GU])
                wg.append((wga, wgb))
            for k in range(KI):
                w = wdn_pool.tile([P, D], BF, tag="wdn", name=f"wd{j}_{k}")
                nc.sync.dma_start(w[:], wdn[j, k * P:(k + 1) * P, :])
                wd.append(w)
            return wg, wd

        wts = {0: load_w(0), 1: load_w(1)}

        # ---- router (fp32): logits^T = gwT.T @ x, transpose back per chunk ----
        lps = []
        for h in range(2):
            pl = ps_s.tile([P, 512], F32, tag="pss", name=f"plT{h}")
            for k in range(KD):
                nc.tensor.matmul(pl[0:E, :], gwts[k][:],
                                 xfs[k][:, h * 512:(h + 1) * 512],
                                 start=(k == 0), stop=(k == KD - 1))
            lps.append(pl)
        lgT = rt_pool.tile([E, T], F32, tag="lgT", bufs=1)
        for h in range(2):
            nc.vector.tensor_copy(lgT[0:E, h * 512:(h + 1) * 512], lps[h][0:E, :])

        cw_tiles, mask_tiles = [], []
        for mt in range(MT):
            pt = ps_s.tile([P, 512], F32, tag="pss", name=f"plt{mt}")
            nc.tensor.transpose(pt[0:P, 0:E], lgT[0:E, mt * P:(mt + 1) * P],
                                idn[0:E, 0:E])
            pl = rt_pool.tile([P, E], F32, tag="pl", bufs=2, name=f"pl{mt}")
            nc.vector.tensor_copy(pl[:], pt[0:P, 0:E])
            m1 = rt_pool.tile([P, 1], F32, tag="m1")
            nc.vector.reduce_max(m1[:], pl[:], axis=AX.X)
            nm1 = rt_pool.tile([P, 1], F32, tag="nm1")
            nc.vector.tensor_scalar(nm1[:], m1[:], -1.0, None, op0=OP.mult)
            t1 = rt_pool.tile([P, E], F32, tag="t1")
            nc.vector.tensor_scalar(t1[:], pl[:], m1[:], None, op0=OP.is_ge)
            lm = rt_pool.tile([P, E], F32, tag="lm")
            nc.vector.scalar_tensor_tensor(lm[:], t1[:], -1e30, pl[:],
                                           op0=OP.mult, op1=OP.add)
            m2 = rt_pool.tile([P, 1], F32, tag="m2")
            nc.vector.reduce_max(m2[:], lm[:], axis=AX.X)
            el = rt_pool.tile([P, E], F32, tag="el")
            nc.scalar.activation(el[:], pl[:], AF.Exp, bias=nm1[:])
            ssum = rt_pool.tile([P, 1], F32, tag="ssum")
            nc.vector.reduce_sum(ssum[:], el[:], axis=AX.X)
            el1 = rt_pool.tile([P, 1], F32, tag="el1")
            nc.vector.reduce_max(el1[:], el[:], axis=AX.X)
            el2 = rt_pool.tile([P, 1], F32, tag="el2")
            nc.scalar.activation(el2[:], m2[:], AF.Exp, bias=nm1[:])
            den = rt_pool.tile([P, 1], F32, tag="den")
            nc.vector.tensor_tensor(den[:], el1[:], el2[:], op=OP.add)
            nc.vector.scalar_tensor_tensor(den[:], ssum[:], 1e-8, den[:],
                                           op0=OP.mult, op1=OP.add)
            rec = rt_pool.tile([P, 1], F32, tag="rec")
            nc.vector.reciprocal(rec[:], den[:])
            msk = rt_pool.tile([P, E], F32R, tag="msk", bufs=MT, name=f"msk{mt}")
            nc.vector.tensor_scalar(msk[:], pl[:], m2[:], None, op0=OP.is_ge)
            cwu = rt_pool.tile([P, E], F32, tag="cwu")
            nc.vector.tensor_tensor(cwu[:], msk[:], el[:], op=OP.mult)
            cw = rt_pool.tile([P, E], F32R, tag="cw", bufs=MT, name=f"cw{mt}")
            nc.vector.tensor_scalar(cw[:], cwu[:], rec[:], None, op0=OP.mult)
            cw_tiles.append(cw)
            mask_tiles.append(msk)

        # ---- exclusive prefix counts R[mt] [P, E] over token order ----
        # (emitted mid-shared-expert so the PE never waits on the DVE softmax)
        r_tiles = []

        def emit_prefix():
            for mt in range(MT):
                pr = ps_s.tile([P, 512], F32, tag="pss", name=f"pr{mt}")
                for mp in range(mt + 1):
                    lhs = tri if mp == mt else one
                    nc.tensor.matmul(pr[0:P, 0:E], lhs[:], mask_tiles[mp][:],
                                     start=(mp == 0), stop=(mp == mt))
                rsb = rt_pool.tile([P, E], F32, tag="rsb", bufs=MT,
                                   name=f"rsb{mt}")
                nc.vector.tensor_copy(rsb[:], pr[0:P, 0:E])
                r_tiles.append(rsb)

        # prep(e): slot->token map + slot gatings (one [2,C] matmul), then
        # transposed dma_gather pulls X_e^T (bf16) directly.
        def prep(e):
            ex = e - 1
            sels = []
            for mt in range(MT):
                rk = rt_pool.tile([P, 1], F32, tag="rk")
                nc.vector.tensor_tensor(rk[:], r_tiles[mt][:, ex:ex + 1],
                                        mask_tiles[mt][:, ex:ex + 1], op=OP.mult)
                rks = rt_pool.tile([P, 1], F32, tag="rks")
                nc.vector.scalar_tensor_tensor(rks[:], mask_tiles[mt][:, ex:ex + 1],
                                               -1.0, rk[:], op0=OP.add, op1=OP.add)
                sl = sel_pool.tile([P, C], F32R, tag="sel", name=f"sel{e}_{mt}")
                nc.vector.tensor_scalar(sl[:], ioc[:], rks[:], None, op0=OP.is_equal)
                sels.append(sl)
            rhs2s = []
            for mt in range(MT):
                r2 = idx_pool.tile([P, 2], F32R, tag="rhs2", bufs=MT,
                                   name=f"r2_{e}_{mt}")
                nc.vector.tensor_copy(r2[:, 0:1], iot[:, mt:mt + 1])
                nc.vector.tensor_copy(r2[:, 1:2], cw_tiles[mt][:, ex:ex + 1])
                rhs2s.append(r2)
            ptc = ps_s.tile([P, 512], F32, tag="pss", name=f"ptc{e}")
            for mt in range(MT):
                nc.tensor.matmul(ptc[0:2, 0:C], rhs2s[mt][:], sels[mt][:],
                                 start=(mt == 0), stop=(mt == MT - 1))
            tokrow = idx_pool.tile([2, C], I16, tag="tokrow", bufs=2, name=f"tok{e}")
            nc.vector.tensor_copy(tokrow[0:2, :], ptc[0:2, 0:C])
            cwrow = idx_pool.tile([2, C], F32, tag="cwrow", bufs=2, name=f"cwr{e}")
            nc.vector.tensor_copy(cwrow[0:2, :], ptc[0:2, 0:C])
            stage = dram_pool.tile([C], I16, tag="idxstage", name=f"ist{e}")
            nc.sync.dma_start(
                bass.AP(tensor=stage.tensor, offset=stage.offset,
                        ap=[[C, 1], [1, C]]), tokrow[0:1, :])
            cstage = dram_pool.tile([C], F32, tag="cwstage", name=f"cst{e}")
            nc.sync.dma_start(
                bass.AP(tensor=cstage.tensor, offset=cstage.offset,
                        ap=[[C, 1], [1, C]]), cwrow[1:2, :])
            idxw = idx_pool.tile([P, C // 16], I16, tag="idxw", name=f"idxw{e}")
            for g in range(8):
                nc.sync.dma_start(
                    idxw[16 * g:16 * (g + 1), :],
                    bass.AP(tensor=stage.tensor, offset=stage.offset,
                            ap=[[1, 16], [16, C // 16]]))
            cws = idx_pool.tile([P, CB], F32, tag="cws", name=f"cws{e}")
            nc.sync.dma_start(cws[:],
                              bass.AP(tensor=cstage.tensor, offset=cstage.offset,
                                      ap=[[1, P], [P, CB]]))
            xet = xet_pool.tile([P, KD, C], BF, tag="xet", name=f"xet{e}")
            nc.gpsimd.dma_gather(xet[:], xTok[:], idxw[:], num_idxs=C,
                                 num_idxs_reg=C, elem_size=D, transpose=True)
            return xet, idxw, cws

        # stage1 over a token block of width w; rhs_fn(k) -> [P, >=w] AP
        def swiglu_block(wg, rhs_fn, w, hooks=None):
            ats = []
            for m in range(KI):
                pg = ps_g.tile([P, 512], F32, tag="psg", name=f"pg{m}")
                for k in range(KD):
                    nc.tensor.matmul(pg[:, 0:w], wg[k][0][:, m * P:(m + 1) * P],
                                     rhs_fn(k, w),
                                     start=(k == 0), stop=(k == KD - 1))
                pu = ps_u.tile([P, 512], F32, tag="psu", name=f"pu{m}")
                for k in range(KD):
                    nc.tensor.matmul(pu[:, 0:w], wg[k][1][:, m * P:(m + 1) * P],
                                     rhs_fn(k, w),
                                     start=(k == 0), stop=(k == KD - 1))
                st = tmp_pool.tile([P, 512], F32, tag="tmp", name=f"st{m}")
                nc.scalar.activation(st[:, 0:w], pg[:, 0:w], AF.Silu)
                at = a_pool.tile([P, 512], BF, tag="a", name=f"a{m}")
                nc.vector.tensor_tensor(at[:, 0:w], st[:, 0:w], pu[:, 0:w],
                                        op=OP.mult)
                ats.append(at)
                if hooks and m in hooks:
                    hooks[m]()
            return ats

        # ---- shared expert (dense over all tokens, direct output writes) ----
        # prefix + the first two preps are emitted mid-stage1 so their PE ops
        # slot in after the DVE softmax has produced the masks.
        wg0, wd0 = wts.pop(0)
        pending = {}

        def emit_prep(e):
            if e < NE and e not in pending:
                pending[e] = prep(e)

        sh_hooks = {2: lambda: (emit_prefix(), emit_prep(1)),
                    5: lambda: emit_prep(2)}
        for tc_i, (off, w) in enumerate(SH_CHUNKS):
            ats = swiglu_block(wg0, lambda k, ww: xbs[k][:, off:off + ww], w,
                               hooks=sh_hooks if tc_i == 0 else None)
            for sub in range(w // P):
                for nd in range(ND2):
                    po = ps_o.tile([P, 512], F32, tag="pso",
                                   name=f"spo{tc_i}_{sub}_{nd}")
                    for k in range(KI):
                        nc.tensor.matmul(po[:], ats[k][:, sub * P:(sub + 1) * P],
                                         wd0[k][:, nd * 512:(nd + 1) * 512],
                                         start=(k == 0), stop=(k == KI - 1))
                    ot = tmp_pool.tile([P, 512], F32, tag="tmp",
                                       name=f"so{tc_i}_{sub}_{nd}")
                    nc.vector.tensor_copy(ot[:], po[:])
                    rows = off + sub * P
                    nc.sync.dma_start(out[rows:rows + P, nd * 512:(nd + 1) * 512],
                                      ot[:])

        # ---- routed experts (weights prefetched 1 ahead, prep 2 ahead) ----
        for e in range(1, NE):
            xet, idxw, cws = pending.pop(e)
            wg, wd = wts.pop(e)
            if e + 1 < NE:
                wts[e + 1] = load_w(e + 1)

            ats = swiglu_block(wg, lambda k, ww: xet[:, k, 0:ww], CC,
                               hooks={3: lambda: emit_prep(e + 2)})

            ysb = y_pool.tile([P, CB, D], F32, tag="y", name=f"y{e}")
            for cb, cw_w in enumerate(CC_CHUNKS):
                for nd in range(ND2):
                    po = ps_o.tile([P, 512], F32, tag="pso",
                                   name=f"po{e}_{cb}_{nd}")
                    for k in range(KI):
                        nc.tensor.matmul(po[0:cw_w, :],
                                         ats[k][:, cb * P:cb * P + cw_w],
                                         wd[k][:, nd * 512:(nd + 1) * 512],
                                         start=(k == 0), stop=(k == KI - 1))
                    nc.vector.tensor_scalar(
                        ysb[0:cw_w, cb, nd * 512:(nd + 1) * 512],
                        po[0:cw_w, :], cws[0:cw_w, cb:cb + 1], None,
                        op0=OP.mult)
                nc.gpsimd.dma_scatter_add(out[:], ysb[:, cb:cb + 1, :],
                                          idxw[:, 8 * cb:8 * cb + cw_w // 16],
                                          num_idxs=cw_w, num_idxs_reg=cw_w,
                                          elem_size=D)

    nc.compile()
    return nc


_NC_CACHE = None


def _get_nc():
    global _NC_CACHE
    if _NC_CACHE is None:
        _NC_CACHE = build_moe()
    return _NC_CACHE


def _prep_weights(gate_weight, shared_gate_up, shared_down,
                  experts_gate_up, experts_down):
    wgu = np.zeros((NE, D, GU), np.float32)
    wgu[0, :, 0:I] = shared_gate_up[0:I].T
    wgu[0, :, IP:IP + I] = shared_gate_up[I:2 * I].T
    for e in range(E):
        wgu[e + 1, :, 0:I] = experts_gate_up[e, 0:I].T
        wgu[e + 1, :, IP:IP + I] = experts_gate_up[e, I:2 * I].T
    wdn = np.zeros((NE, IP, D), np.float32)
    wdn[0, 0:I, :] = shared_down.T
    for e in range(E):
        wdn[e + 1, 0:I, :] = experts_down[e].T
    gwT = np.ascontiguousarray(gate_weight.T.astype(np.float32))
    BF_NP = ml_dtypes.bfloat16
    return gwT, wgu.astype(BF_NP), wdn.astype(BF_NP)


def _consts():
    return {
        "ident": np.eye(P, dtype=np.float32),
        "triu": np.triu(np.ones((P, P), np.float32), 1),
        "ones": np.ones((P, P), np.float32),
        "iotaC": np.broadcast_to(np.arange(C, dtype=np.float32), (P, C)).copy(),
        "iotaT": np.arange(T, dtype=np.float32),
    }


def make_in_maps(hidden_states, gate_weight, shared_gate_up, shared_down,
                 experts_gate_up, experts_down):
    hidden_states = np.asarray(hidden_states, dtype=np.float32)
    x = hidden_states.reshape(B * S, D)
    gwT, wgu, wdn = _prep_weights(
        np.asarray(gate_weight, np.float32),
        np.asarray(shared_gate_up, np.float32),
        np.asarray(shared_down, np.float32),
        np.asarray(experts_gate_up, np.float32),
        np.asarray(experts_down, np.float32))
    consts = _consts()
    BF_NP = ml_dtypes.bfloat16
    in_maps = []
    for c in range(N_CORES):
        xs = np.ascontiguousarray(x[c * T:(c + 1) * T])
        xsT = np.ascontiguousarray(xs.T)
        in_maps.append({
            "xTf": xsT, "xTok": xs.astype(BF_NP),
            "gwT": gwT, "wgu": wgu, "wdn": wdn, **consts,
        })
    return in_maps


def kernel(hidden_states, gate_weight, shared_gate_up, shared_down,
           experts_gate_up, experts_down):
    in_maps = make_in_maps(hidden_states, gate_weight, shared_gate_up,
                           shared_down, experts_gate_up, experts_down)
    nc = _get_nc()
    res = run_bass_kernel_spmd(nc, in_maps, core_ids=list(range(N_CORES)))
    out = np.concatenate([res.results[c]["out"] for c in range(N_CORES)], axis=0)
    return out.reshape(B, S, D)
